# revision 8
# baseline (speedup 1.0000x reference)
"""Trainium2 Bass kernel for nn_CNNVectorForm (LeNet-style CNN, batch 8192).

Pipeline per core (data-parallel over batch, 1024 images/core):
  conv 5x5 VALID (1->20ch, 28->24) -> 2x2 maxpool -> fc1(2880->500) + relu
  -> fc2(500->10) + softmax

Device formulation (v3 — bf16, coalesced DMA, dense+warm PE):
  * Everything feature-major [features, batch]; batch rides the free dim
    (nb=256 per tile, 4 tiles per core).
  * Inputs, conv/fc weights in bf16 (validated ~4.6e-3 final rel err);
    PSUM accumulation stays fp32.
  * Conv as Toeplitz matmul with K=128 = 8 input rows x 16 cols gathers:
    host im2col expansion is only 1.96x (3.1MB/core in bf16), loaded as
    13 contiguous per-partition DMAs (128 descriptors each) instead of
    96 small gathers - the baseline's DMA-issue serialization starved
    the PE below 1.2GHz.
  * One 8-row gather feeds 4 conv rows = 2 pooled-row "sets"; a set is
    4 matmuls (2 conv rows x 2 col-parities) into 2 one-bank PSUM tiles
    [120, 2, 256] packed so 2x2 maxpool is: scalar copy (PSUM->SBUF),
    DVE max (SBUF,PSUM), DVE bf16 max (halves) -> a1 bf16.  Every 6th
    set swaps work between scalar and DVE to balance both at ~95us.
  * fc1 runs 1:1 interleaved with conv (4+4 matmuls per set, lag 3) so
    the PE never idles; dummy warm-up matmuls during the ~7us framework
    preamble bring the PE HAM clock to 2.4GHz before real work starts.
  * fc1 accumulators: 2 one-bank PSUM tiles [125, 2, 256]; start/stop
    flags are bank-granular (has_written clears whole banks).
  * conv bias folded into the fc1 bias on the host.
  * The relu/fc2/softmax tail of each batch tile is staggered across
    the first sets of the next tile so it hides under conv/fc1; softmax
    row-sum via an all-ones [10,10] matmul, fast-approx reciprocal;
    output [10, npc] is transposed on the host.
  * DMA rings: sync (HWDGE) carries t8 + input; scalar carries the tiny
    biases/fc2; gpsimd (SWDGE) streams the fc1 weights.
"""

import numpy as np

N, H, W = 8192, 28, 28
COUT, KS = 20, 5
NCORES = 8
NPC = N // NCORES        # images per core
G = 6                    # row groups of 8 input rows (stride 4)
JB = 2                   # 16-wide column blocks at offsets 0, 12
NSEG = JB * G            # 12 (jb, g) segments -> 24 pooled-feature blocks
FC1_IN, FC1_OUT, FC2_OUT = 2880, 500, 10
MT, MTS = 4, 125         # fc1 M tiles
KB, KBS = 24, 120        # fc1 K blocks (one per (pooled row, column half))
LAG = 3                  # fc1 trails conv by this many sets
NWARM = 18               # PE warm-up matmuls

_cache = {}


def _build(npc, nb):
    from contextlib import ExitStack

    import concourse.tile as tile
    from concourse import bacc, mybir

    f32 = mybir.dt.float32
    bf16 = mybir.dt.bfloat16
    nbt = npc // nb

    nc = bacc.Bacc(
        "TRN2",
        target_bir_lowering=False,
        debug=False,
        enable_asserts=False,
        num_devices=NCORES,
    )

    # host-im2col'd input: xg[p, jb, g, b] = x[(4g + p//16)*28 + 12jb + p%16, b]
    xg_d = nc.dram_tensor("xg", [128, JB, G, npc], bf16, kind="ExternalInput").ap()
    t8_d = nc.dram_tensor("t8", [128, 8 * KBS], bf16, kind="ExternalInput").ap()
    w1_d = nc.dram_tensor("w1", [KBS, KB * FC1_OUT], bf16, kind="ExternalInput").ap()
    b1_d = nc.dram_tensor("b1", [MTS, MT], f32, kind="ExternalInput").ap()
    w2_d = nc.dram_tensor("w2", [MTS, MT * FC2_OUT], bf16, kind="ExternalInput").ap()
    b2_d = nc.dram_tensor("b2", [FC2_OUT, 1], f32, kind="ExternalInput").ap()
    o_d = nc.dram_tensor("out", [FC2_OUT, npc], f32, kind="ExternalOutput").ap()

    with tile.TileContext(nc) as tc, ExitStack() as ctx:
        const = ctx.enter_context(tc.tile_pool(name="const", bufs=1))
        spool = ctx.enter_context(tc.tile_pool(name="spool", bufs=3))
        mpool = ctx.enter_context(tc.tile_pool(name="mpool", bufs=3))
        a1pool = ctx.enter_context(tc.tile_pool(name="a1", bufs=6))
        a2pool = ctx.enter_context(tc.tile_pool(name="a2", bufs=8))
        smpool = ctx.enter_context(tc.tile_pool(name="softmax", bufs=2))
        cpsum = ctx.enter_context(tc.tile_pool(name="cpsum", bufs=4, space="PSUM"))
        fpsum = ctx.enter_context(tc.tile_pool(name="fpsum", bufs=1, space="PSUM"))
        tpsum = ctx.enter_context(tc.tile_pool(name="tpsum", bufs=1, space="PSUM"))

        # --- PE warm-up: HAM releases the 2.4GHz clock only after ~3.4us of
        # sustained matmul activity; burn the framework preamble + first DMA
        # wait on dummies so the real stream starts warm.
        dums = const.tile([128, 384], bf16)
        nc.gpsimd.memset(dums[:], 0.0)
        warmps = tpsum.tile([128, nb], f32, tag="p2f", name="warmps")
        for wi in range(NWARM):
            nc.tensor.matmul(
                warmps[:, :nb], dums[:, 0:128], dums[:, 128 : 128 + nb],
                start=True, stop=True,
            )

        # --- constants / weights -------------------------------------------
        # sync ring: conv stationaries first, then a small first input chunk
        # so conv can start ~10.5us in, then the 12 input segments.
        t8s = const.tile([128, 8 * KBS], bf16)
        nc.sync.dma_start(t8s[:], t8_d[:])
        xfirst = const.tile([128, nb], bf16)
        nc.sync.dma_start(xfirst[:], xg_d[:, 0, 0, 0:nb])
        xseg = [None] * NSEG
        for i in range(NSEG):
            jb, g = i // G, i % G
            xseg[i] = const.tile([128, npc], bf16, name=f"xs{i}")
        # input segments split across two HWDGE rings (~133GB/s each):
        # even on sync, first odd ones on scalar; the last three odd-seg
        # issues are deferred into the loop so the scalar engine is free
        # for the first pooling copies.
        for i in range(0, NSEG, 2):
            nc.sync.dma_start(xseg[i][:], xg_d[:, i // G, i % G, :])
        b1t = const.tile([MTS, MT], f32)
        nc.scalar.dma_start(b1t[:], b1_d[:])
        w2t = const.tile([MTS, MT * FC2_OUT], bf16)
        nc.scalar.dma_start(w2t[:], w2_d[:])
        b2t = const.tile([FC2_OUT, 1], f32)
        nc.scalar.dma_start(b2t[:], b2_d[:])
        for i in (1, 3, 5):
            nc.scalar.dma_start(xseg[i][:], xg_d[:, i // G, i % G, :])
        deferred_segs = {2: 7, 6: 9, 10: 11}  # sidx -> seg to issue there
        # gpsimd (SWDGE) ring: the fc1 weight stream; a small first chunk so
        # fc1 j=0 isn't gated on a big transfer.
        w1t = []       # per j-block [KBS, FC1_OUT] views
        w1chunks = [2, 4, 4, 4, 4, 4, 2]
        j0 = 0
        for ci, cn in enumerate(w1chunks):
            wt = const.tile([KBS, cn * FC1_OUT], bf16, name=f"w1c{ci}")
            nc.gpsimd.dma_start(
                wt[:], w1_d[:, j0 * FC1_OUT : (j0 + cn) * FC1_OUT]
            )
            for k in range(cn):
                w1t.append(wt[:, k * FC1_OUT : (k + 1) * FC1_OUT])
            j0 += cn

        ones10 = const.tile([FC2_OUT, FC2_OUT], bf16)
        nc.gpsimd.memset(ones10[:], 1.0)
        # warm the scalar activation table set (exp+relu+copy) while idle
        wout = const.tile([1, 8], f32)
        nc.scalar.activation(wout[:], dums[:1, :8], mybir.ActivationFunctionType.Exp)

        outbuf = const.tile([FC2_OUT, npc], f32)

        def w1_slice(j, mt):
            return w1t[j][:, mt * MTS : (mt + 1) * MTS]

        def fc1_mms(fp, a1, j):
            # fp = (fpA, fpB): two 1-bank tiles, each packing 2 M-tiles.
            # start/stop are bank-wide (has_written clears the whole bank),
            # so only the first/last matmul touching a bank carries the flag.
            for mt in range(MT):
                nc.tensor.matmul(
                    fp[mt // 2][:, mt % 2, :],
                    w1_slice(j, mt),
                    a1[j][:],
                    start=(j == 0 and mt % 2 == 0),
                    stop=(j == KB - 1 and mt % 2 == 1),
                )

        def tail_slots(bt, fp, b0):
            """relu -> fc2 -> softmax for batch tile bt, split into slots
            emitted across the first sets of the next batch tile.  The relu
            slots (0,1) must be emitted before the next tile's fc1 j=0 (the
            fpA/fpB slots are reused, bufs=1)."""
            a2t = [None] * MT
            st8 = {}

            def relu_pair(k):
                def f():
                    for mt in (2 * k, 2 * k + 1):
                        a2 = a2pool.tile(
                            [MTS, nb], bf16, tag="a2", name=f"a2_{bt}_{mt}"
                        )
                        src = fp[mt // 2][:, mt % 2, :]
                        if mt % 2 == 0:
                            nc.scalar.activation(
                                a2[:], src, mybir.ActivationFunctionType.Relu,
                                bias=b1t[:, mt : mt + 1],
                            )
                        else:
                            nc.vector.tensor_scalar(
                                a2[:], src, b1t[:, mt : mt + 1], 0.0,
                                mybir.AluOpType.add, mybir.AluOpType.max,
                            )
                        a2t[mt] = a2
                return f

            def fc2():
                p2f = tpsum.tile([FC2_OUT, nb], f32, tag="p2f", name=f"p2f_{bt}")
                for mt in range(MT):
                    nc.tensor.matmul(
                        p2f[:],
                        w2t[:, mt * FC2_OUT : (mt + 1) * FC2_OUT],
                        a2t[mt][:],
                        start=(mt == 0),
                        stop=(mt == MT - 1),
                    )
                st8["p2f"] = p2f

            def expsum():
                e = smpool.tile([FC2_OUT, nb], bf16, tag="e", name=f"e_{bt}")
                nc.scalar.activation(
                    e[:], st8["p2f"][:], mybir.ActivationFunctionType.Exp,
                    bias=b2t[:, 0:1],
                )
                ssum = tpsum.tile([FC2_OUT, nb], f32, tag="ssum", name=f"ss_{bt}")
                nc.tensor.matmul(ssum[:], ones10[:], e[:], start=True, stop=True)
                st8["e"], st8["ssum"] = e, ssum

            def norm():
                rinv = smpool.tile([FC2_OUT, nb], f32, tag="ri", name=f"ri_{bt}")
                nc.vector.reciprocal_approx_fast(rinv[:], st8["ssum"][:])
                nc.vector.tensor_mul(outbuf[:, b0 : b0 + nb], st8["e"][:], rinv[:])
                nc.sync.dma_start(o_d[:, b0 : b0 + nb], outbuf[:, b0 : b0 + nb])

            return [relu_pair(0), relu_pair(1), fc2, expsum, norm]

        pending = []
        for bt in range(nbt):
            b0 = bt * nb
            a1 = [None] * KB
            fp = (
                fpsum.tile([MTS, 2, nb], f32, tag="fpA", name=f"fpA{bt}"),
                fpsum.tile([MTS, 2, nb], f32, tag="fpB", name=f"fpB{bt}"),
            )
            for idx in range(NSEG):
                for st in range(2):
                    sidx = idx * 2 + st
                    if bt == 0 and sidx == 0:
                        rhs = xfirst[:]
                    else:
                        rhs = xseg[idx][:, b0 : b0 + nb]
                    # conv: 4 matmuls -> 2 one-bank PSUM tiles [120, 2, nb]
                    # t0 = conv row 4g+2st (both col parities), t1 = row +1
                    t0 = cpsum.tile([KBS, 2, nb], f32, tag="cps", name=f"t0_{bt}_{sidx}")
                    t1 = cpsum.tile([KBS, 2, nb], f32, tag="cps", name=f"t1_{bt}_{sidx}")
                    for half, tt in ((0, t0), (1, t1)):
                        dd = 2 * st + half
                        for eo in range(2):
                            nc.tensor.matmul(
                                tt[:, eo, :],
                                t8s[:, (dd * 2 + eo) * KBS : (dd * 2 + eo + 1) * KBS],
                                rhs,
                                start=(eo == 0),
                                stop=(eo == 1),
                            )
                    # 2x2 maxpool: max over (t0, t1) x (eo halves).  PSUM
                    # evacuation is split between scalar and DVE; every 6th
                    # set uses 2 scalar copies + a cheap bf16 DVE max instead
                    # of 1 copy + a slow fp32-PSUM DVE max, balancing the
                    # two engines.
                    m = mpool.tile([KBS, 2, nb], bf16, tag="m")
                    if sidx % 6 == 5:
                        s0 = spool.tile([KBS, 2, nb], bf16, tag="s")
                        nc.scalar.copy(s0[:], t0[:])
                        s1 = spool.tile([KBS, 2, nb], bf16, tag="s")
                        nc.scalar.copy(s1[:], t1[:])
                        nc.vector.tensor_max(m[:], s0[:], s1[:])
                    else:
                        s = spool.tile([KBS, 2, nb], f32, tag="sf")
                        nc.scalar.copy(s[:], t0[:])
                        nc.vector.tensor_max(m[:], s[:], t1[:])
                    ab = a1pool.tile([KBS, nb], bf16, tag="a1")
                    nc.vector.tensor_max(ab[:], m[:, 0, :], m[:, 1, :])
                    a1[sidx] = ab
                    if bt == 0 and sidx in deferred_segs:
                        i2 = deferred_segs[sidx]
                        nc.scalar.dma_start(
                            xseg[i2][:], xg_d[:, i2 // G, i2 % G, :]
                        )
                    # fc1 trails conv so the PE alternates 4 conv + 4 fc1
                    if sidx >= LAG:
                        fc1_mms(fp, a1, sidx - LAG)
                    # previous tile's relu/fc2/softmax, staggered over sets
                    # 1..5 (relu slots land before this tile's fc1 j=0)
                    if pending and 1 <= sidx <= len(pending):
                        pending[sidx - 1]()
                        if sidx == len(pending):
                            pending = []
            for j in range(KB - LAG, KB):
                fc1_mms(fp, a1, j)
            pending = tail_slots(bt, fp, b0)
        for f in pending:
            f()

    nc.compile()
    return nc


def _prep_weights(conv_w, conv_b, fc1_w, fc1_b, fc2_w, fc2_b):
    import ml_dtypes

    bf16 = ml_dtypes.bfloat16
    conv_w = np.asarray(conv_w, np.float32).reshape(COUT, KS, KS)
    conv_b = np.asarray(conv_b, np.float32)
    fc1_w = np.asarray(fc1_w, np.float32)
    fc1_b = np.asarray(fc1_b, np.float32)
    fc2_w = np.asarray(fc2_w, np.float32)
    fc2_b = np.asarray(fc2_b, np.float32)

    # conv stationaries: T8[p = di8*16+jjp, k = dd*2+eo, m = c*6+q]
    #  = conv_w[c, di8-dd, jjp-(2q+eo)] where both offsets are in [0, 5)
    T8 = np.zeros((128, 8, KBS), np.float32)
    for dd in range(4):
        for eo in range(2):
            for c in range(COUT):
                for q in range(6):
                    jj = 2 * q + eo
                    for di in range(KS):
                        for dj in range(KS):
                            T8[(dd + di) * 16 + jj + dj, dd * 2 + eo, c * 6 + q] = (
                                conv_w[c, di, dj]
                            )
    t8 = np.ascontiguousarray(T8.reshape(128, 8 * KBS)).astype(bf16)

    # fc1 weights to pooled-feature order: block j=(jb*6+g)*2+st, row c*6+q
    # -> original flat feature c*144 + (2g+st)*12 + jb*6 + q
    jv = np.arange(KB)
    jbv, gv, stv = jv // 12, (jv % 12) // 2, jv % 2
    ipv = 2 * gv + stv
    ml = np.arange(KBS)
    cv, qv = ml // 6, ml % 6
    fidx = (
        cv[None, :] * 144 + ipv[:, None] * 12 + jbv[:, None] * 6 + qv[None, :]
    )  # [KB, KBS]
    w1 = fc1_w[:, fidx]                       # [500, KB, KBS]
    w1 = np.ascontiguousarray(w1.transpose(2, 1, 0)).reshape(KBS, KB * FC1_OUT)
    w1 = w1.astype(bf16)

    # conv bias folded into fc1 bias (maxpool commutes with per-channel const)
    cb_vec = np.repeat(conv_b, 144)
    b1p = fc1_b + fc1_w @ cb_vec
    b1 = np.ascontiguousarray(b1p.reshape(MT, MTS).T)

    w2 = np.ascontiguousarray(
        fc2_w.T.reshape(MT, MTS, FC2_OUT).transpose(1, 0, 2)
    ).reshape(MTS, MT * FC2_OUT).astype(bf16)
    b2 = np.ascontiguousarray(fc2_b.reshape(FC2_OUT, 1))
    return t8, w1, b1, w2, b2


# im2col pixel indices: IDX[p, jb, g] = (4g + p//16)*28 + 12jb + (p%16)
_P = np.arange(128)
_IDX = (
    (4 * np.arange(G)[None, None, :] + (_P // 16)[:, None, None]) * W
    + 12 * np.arange(JB)[None, :, None]
    + (_P % 16)[:, None, None]
)  # [128, JB, G]


def _prep_x(x_bf16):
    """x_bf16 [npc, 784] bf16 -> xg [128, JB, G, npc] bf16."""
    g = x_bf16[:, _IDX.reshape(-1)]           # [npc, 128*JB*G]
    g = g.T.reshape(128, JB, G, x_bf16.shape[0])
    return np.ascontiguousarray(g)


def _run(inputs, npc=NPC, nb=256, trace=False):
    import ml_dtypes
    from concourse import bass_utils

    key = (npc, nb)
    if key not in _cache:
        _cache[key] = _build(npc, nb)
    nc = _cache[key]

    t8, w1, b1, w2, b2 = _prep_weights(
        inputs["conv_w"], inputs["conv_b"], inputs["fc1_w"],
        inputs["fc1_b"], inputs["fc2_w"], inputs["fc2_b"],
    )
    x = np.asarray(inputs["x"], np.float32).reshape(-1, H * W)
    n_total = x.shape[0]
    assert n_total == NCORES * npc
    xb = x.astype(ml_dtypes.bfloat16)

    in_maps = [
        {
            "xg": _prep_x(xb[i * npc : (i + 1) * npc]),
            "t8": t8, "w1": w1, "b1": b1, "w2": w2, "b2": b2,
        }
        for i in range(NCORES)
    ]
    res = bass_utils.run_bass_kernel_spmd(
        nc, in_maps, core_ids=list(range(NCORES)), trace=trace
    )
    out = np.concatenate(
        [np.asarray(res.results[i]["out"], np.float32).T for i in range(NCORES)],
        axis=0,
    )
    return out, res


def kernel(**inputs):
    out, _ = _run(inputs)
    return out


# revision 9
# speedup vs baseline: 1.0874x; 1.0874x over previous
"""Trainium2 Bass kernel for nn_CNNVectorForm (LeNet-style CNN, batch 8192).

Pipeline per core (data-parallel over batch, 1024 images/core):
  conv 5x5 VALID (1->20ch, 28->24) -> 2x2 maxpool -> fc1(2880->500) + relu
  -> fc2(500->10) + softmax

Device formulation (v3 — bf16, coalesced DMA, dense+warm PE):
  * Everything feature-major [features, batch]; batch rides the free dim
    (nb=256 per tile, 4 tiles per core).
  * Inputs, conv/fc weights in bf16 (validated ~4.6e-3 final rel err);
    PSUM accumulation stays fp32.
  * Conv as Toeplitz matmul with K=128 = 8 input rows x 16 cols gathers:
    host im2col expansion is only 1.96x (3.1MB/core in bf16), loaded as
    13 contiguous per-partition DMAs (128 descriptors each) instead of
    96 small gathers - the baseline's DMA-issue serialization starved
    the PE below 1.2GHz.
  * One 8-row gather feeds 4 conv rows = 2 pooled-row "sets"; a set is
    4 matmuls (2 conv rows x 2 col-parities) into 2 one-bank PSUM tiles
    [120, 2, 256] packed so 2x2 maxpool is: scalar copy (PSUM->SBUF),
    DVE max (SBUF,PSUM), DVE bf16 max (halves) -> a1 bf16.  Every 6th
    set swaps work between scalar and DVE to balance both at ~95us.
  * fc1 runs 1:1 interleaved with conv (4+4 matmuls per set, lag 3) so
    the PE never idles; dummy warm-up matmuls during the ~7us framework
    preamble bring the PE HAM clock to 2.4GHz before real work starts.
  * fc1 accumulators: 2 one-bank PSUM tiles [125, 2, 256]; start/stop
    flags are bank-granular (has_written clears whole banks).
  * conv bias folded into the fc1 bias on the host.
  * The relu/fc2/softmax tail of each batch tile is staggered across
    the first sets of the next tile so it hides under conv/fc1; softmax
    row-sum via an all-ones [10,10] matmul, fast-approx reciprocal;
    output [10, npc] is transposed on the host.
  * DMA rings: sync (HWDGE) carries t8 + input; scalar carries the tiny
    biases/fc2; gpsimd (SWDGE) streams the fc1 weights.
"""

import numpy as np

N, H, W = 8192, 28, 28
COUT, KS = 20, 5
NCORES = 8
NPC = N // NCORES        # images per core
G = 6                    # row groups of 8 input rows (stride 4)
JB = 2                   # 16-wide column blocks at offsets 0, 12
NSEG = JB * G            # 12 (jb, g) segments -> 24 pooled-feature blocks
FC1_IN, FC1_OUT, FC2_OUT = 2880, 500, 10
MT, MTS = 4, 125         # fc1 M tiles
KB, KBS = 24, 120        # fc1 K blocks (one per (pooled row, column half))
LAG = 3                  # fc1 trails conv by this many sets
NWARM = 18               # PE warm-up matmuls

_cache = {}


def _build(npc, nb):
    from contextlib import ExitStack

    import concourse.tile as tile
    from concourse import bacc, mybir

    f32 = mybir.dt.float32
    bf16 = mybir.dt.bfloat16
    nbt = npc // nb

    nc = bacc.Bacc(
        "TRN2",
        target_bir_lowering=False,
        debug=False,
        enable_asserts=False,
        num_devices=NCORES,
    )

    # host-im2col'd input: xg[p, jb, g, b] = x[(4g + p//16)*28 + 12jb + p%16, b]
    xg_d = nc.dram_tensor(
        "xg", [128, nbt, NSEG, nb], bf16, kind="ExternalInput"
    ).ap()
    t8_d = nc.dram_tensor("t8", [128, 8 * KBS], bf16, kind="ExternalInput").ap()
    w1_d = nc.dram_tensor("w1", [KBS, KB * FC1_OUT], bf16, kind="ExternalInput").ap()
    b1_d = nc.dram_tensor("b1", [MTS, MT], f32, kind="ExternalInput").ap()
    w2_d = nc.dram_tensor("w2", [MTS, MT * FC2_OUT], bf16, kind="ExternalInput").ap()
    b2_d = nc.dram_tensor("b2", [FC2_OUT, 1], f32, kind="ExternalInput").ap()
    o_d = nc.dram_tensor("out", [FC2_OUT, npc], f32, kind="ExternalOutput").ap()

    with tile.TileContext(nc) as tc, ExitStack() as ctx:
        const = ctx.enter_context(tc.tile_pool(name="const", bufs=1))
        spool = ctx.enter_context(tc.tile_pool(name="spool", bufs=3))
        mpool = ctx.enter_context(tc.tile_pool(name="mpool", bufs=3))
        a1pool = ctx.enter_context(tc.tile_pool(name="a1", bufs=6))
        a2pool = ctx.enter_context(tc.tile_pool(name="a2", bufs=8))
        smpool = ctx.enter_context(tc.tile_pool(name="softmax", bufs=2))
        cpsum = ctx.enter_context(tc.tile_pool(name="cpsum", bufs=4, space="PSUM"))
        fpsum = ctx.enter_context(tc.tile_pool(name="fpsum", bufs=1, space="PSUM"))
        tpsum = ctx.enter_context(tc.tile_pool(name="tpsum", bufs=1, space="PSUM"))

        # --- PE warm-up: HAM releases the 2.4GHz clock only after ~3.4us of
        # sustained matmul activity; burn the framework preamble + first DMA
        # wait on dummies so the real stream starts warm.
        dums = const.tile([128, 384], bf16)
        nc.gpsimd.memset(dums[:], 0.0)
        warmps = tpsum.tile([128, nb], f32, tag="p2f", name="warmps")
        for wi in range(NWARM):
            nc.tensor.matmul(
                warmps[:, :nb], dums[:, 0:128], dums[:, 128 : 128 + nb],
                start=True, stop=True,
            )

        # --- constants / weights -------------------------------------------
        # sync ring: conv stationaries first, then a small first input chunk
        # so conv can start ~10.5us in, then the 12 input segments.
        t8s = const.tile([128, 8 * KBS], bf16)
        nc.sync.dma_start(t8s[:], t8_d[:])
        xfirst = const.tile([128, nb], bf16)
        nc.sync.dma_start(xfirst[:], xg_d[:, 0, 0, :])
        # input: batch-major layout -> one DMA per batch tile with 6KB
        # per-partition contiguous runs (~300GB/s); bt0's whole input is
        # resident by ~11.5us, later tiles stream far ahead of use.
        xbt = []
        for b in range(nbt):
            xt = const.tile([128, NSEG * nb], bf16, name=f"xb{b}")
            nc.sync.dma_start(xt[:], xg_d[:, b, :, :])
            xbt.append(xt)
        b1t = const.tile([MTS, MT], f32)
        nc.scalar.dma_start(b1t[:], b1_d[:])
        w2t = const.tile([MTS, MT * FC2_OUT], bf16)
        nc.scalar.dma_start(w2t[:], w2_d[:])
        b2t = const.tile([FC2_OUT, 1], f32)
        nc.scalar.dma_start(b2t[:], b2_d[:])
        # gpsimd (SWDGE) ring: the fc1 weight stream; a small first chunk so
        # fc1 j=0 isn't gated on a big transfer.
        w1t = []       # per j-block [KBS, FC1_OUT] views
        w1chunks = [2, 4, 4, 4, 4, 4, 2]
        j0 = 0
        for ci, cn in enumerate(w1chunks):
            wt = const.tile([KBS, cn * FC1_OUT], bf16, name=f"w1c{ci}")
            nc.gpsimd.dma_start(
                wt[:], w1_d[:, j0 * FC1_OUT : (j0 + cn) * FC1_OUT]
            )
            for k in range(cn):
                w1t.append(wt[:, k * FC1_OUT : (k + 1) * FC1_OUT])
            j0 += cn

        ones10 = const.tile([FC2_OUT, FC2_OUT], bf16)
        nc.gpsimd.memset(ones10[:], 1.0)
        # warm the scalar activation table set (exp+relu+copy) while idle
        wout = const.tile([1, 8], f32)
        nc.scalar.activation(wout[:], dums[:1, :8], mybir.ActivationFunctionType.Exp)

        outbuf = const.tile([FC2_OUT, npc], f32)

        def w1_slice(j, mt):
            return w1t[j][:, mt * MTS : (mt + 1) * MTS]

        def fc1_mms(fp, a1, j):
            # fp = (fpA, fpB): two 1-bank tiles, each packing 2 M-tiles.
            # start/stop are bank-wide (has_written clears the whole bank),
            # so only the first/last matmul touching a bank carries the flag.
            for mt in range(MT):
                nc.tensor.matmul(
                    fp[mt // 2][:, mt % 2, :],
                    w1_slice(j, mt),
                    a1[j][:],
                    start=(j == 0 and mt % 2 == 0),
                    stop=(j == KB - 1 and mt % 2 == 1),
                )

        def tail_slots(bt, fp, b0):
            """relu -> fc2 -> softmax for batch tile bt, split into slots
            emitted across the first sets of the next batch tile.  The relu
            slots (0,1) must be emitted before the next tile's fc1 j=0 (the
            fpA/fpB slots are reused, bufs=1)."""
            a2t = [None] * MT
            st8 = {}

            def relu_pair(k):
                def f():
                    for mt in (2 * k, 2 * k + 1):
                        a2 = a2pool.tile(
                            [MTS, nb], bf16, tag="a2", name=f"a2_{bt}_{mt}"
                        )
                        src = fp[mt // 2][:, mt % 2, :]
                        if mt % 2 == 0:
                            nc.scalar.activation(
                                a2[:], src, mybir.ActivationFunctionType.Relu,
                                bias=b1t[:, mt : mt + 1],
                            )
                        else:
                            nc.vector.tensor_scalar(
                                a2[:], src, b1t[:, mt : mt + 1], 0.0,
                                mybir.AluOpType.add, mybir.AluOpType.max,
                            )
                        a2t[mt] = a2
                return f

            def fc2():
                p2f = tpsum.tile([FC2_OUT, nb], f32, tag="p2f", name=f"p2f_{bt}")
                for mt in range(MT):
                    nc.tensor.matmul(
                        p2f[:],
                        w2t[:, mt * FC2_OUT : (mt + 1) * FC2_OUT],
                        a2t[mt][:],
                        start=(mt == 0),
                        stop=(mt == MT - 1),
                    )
                st8["p2f"] = p2f

            def expsum():
                e = smpool.tile([FC2_OUT, nb], bf16, tag="e", name=f"e_{bt}")
                nc.scalar.activation(
                    e[:], st8["p2f"][:], mybir.ActivationFunctionType.Exp,
                    bias=b2t[:, 0:1],
                )
                ssum = tpsum.tile([FC2_OUT, nb], f32, tag="ssum", name=f"ss_{bt}")
                nc.tensor.matmul(ssum[:], ones10[:], e[:], start=True, stop=True)
                st8["e"], st8["ssum"] = e, ssum

            def norm():
                rinv = smpool.tile([FC2_OUT, nb], f32, tag="ri", name=f"ri_{bt}")
                nc.vector.reciprocal_approx_fast(rinv[:], st8["ssum"][:])
                nc.vector.tensor_mul(outbuf[:, b0 : b0 + nb], st8["e"][:], rinv[:])
                nc.sync.dma_start(o_d[:, b0 : b0 + nb], outbuf[:, b0 : b0 + nb])

            return [relu_pair(0), relu_pair(1), fc2, expsum, norm]

        pending = []
        for bt in range(nbt):
            b0 = bt * nb
            a1 = [None] * KB
            fp = (
                fpsum.tile([MTS, 2, nb], f32, tag="fpA", name=f"fpA{bt}"),
                fpsum.tile([MTS, 2, nb], f32, tag="fpB", name=f"fpB{bt}"),
            )
            for idx in range(NSEG):
                for st in range(2):
                    sidx = idx * 2 + st
                    if bt == 0 and idx == 0:
                        rhs = xfirst[:]
                    else:
                        rhs = xbt[bt][:, idx * nb : (idx + 1) * nb]
                    # conv: 4 matmuls -> 2 one-bank PSUM tiles [120, 2, nb]
                    # t0 = conv row 4g+2st (both col parities), t1 = row +1
                    t0 = cpsum.tile([KBS, 2, nb], f32, tag="cps", name=f"t0_{bt}_{sidx}")
                    t1 = cpsum.tile([KBS, 2, nb], f32, tag="cps", name=f"t1_{bt}_{sidx}")
                    for half, tt in ((0, t0), (1, t1)):
                        dd = 2 * st + half
                        for eo in range(2):
                            nc.tensor.matmul(
                                tt[:, eo, :],
                                t8s[:, (dd * 2 + eo) * KBS : (dd * 2 + eo + 1) * KBS],
                                rhs,
                                start=(eo == 0),
                                stop=(eo == 1),
                            )
                    # 2x2 maxpool: max over (t0, t1) x (eo halves).  PSUM
                    # evacuation is split between scalar and DVE; every 6th
                    # set uses 2 scalar copies + a cheap bf16 DVE max instead
                    # of 1 copy + a slow fp32-PSUM DVE max, balancing the
                    # two engines.
                    m = mpool.tile([KBS, 2, nb], bf16, tag="m")
                    if sidx % 6 == 5:
                        s0 = spool.tile([KBS, 2, nb], bf16, tag="s")
                        nc.scalar.copy(s0[:], t0[:])
                        s1 = spool.tile([KBS, 2, nb], bf16, tag="s")
                        nc.scalar.copy(s1[:], t1[:])
                        nc.vector.tensor_max(m[:], s0[:], s1[:])
                    else:
                        s = spool.tile([KBS, 2, nb], f32, tag="sf")
                        nc.scalar.copy(s[:], t0[:])
                        nc.vector.tensor_max(m[:], s[:], t1[:])
                    ab = a1pool.tile([KBS, nb], bf16, tag="a1")
                    nc.vector.tensor_max(ab[:], m[:, 0, :], m[:, 1, :])
                    a1[sidx] = ab
                    # fc1 trails conv so the PE alternates 4 conv + 4 fc1
                    if sidx >= LAG:
                        fc1_mms(fp, a1, sidx - LAG)
                    # previous tile's relu/fc2/softmax, staggered over sets
                    # 1..5 (relu slots land before this tile's fc1 j=0)
                    if pending and 1 <= sidx <= len(pending):
                        pending[sidx - 1]()
                        if sidx == len(pending):
                            pending = []
            for j in range(KB - LAG, KB):
                fc1_mms(fp, a1, j)
            pending = tail_slots(bt, fp, b0)
        for f in pending:
            f()

    nc.compile()
    return nc


def _prep_weights(conv_w, conv_b, fc1_w, fc1_b, fc2_w, fc2_b):
    import ml_dtypes

    bf16 = ml_dtypes.bfloat16
    conv_w = np.asarray(conv_w, np.float32).reshape(COUT, KS, KS)
    conv_b = np.asarray(conv_b, np.float32)
    fc1_w = np.asarray(fc1_w, np.float32)
    fc1_b = np.asarray(fc1_b, np.float32)
    fc2_w = np.asarray(fc2_w, np.float32)
    fc2_b = np.asarray(fc2_b, np.float32)

    # conv stationaries: T8[p = di8*16+jjp, k = dd*2+eo, m = c*6+q]
    #  = conv_w[c, di8-dd, jjp-(2q+eo)] where both offsets are in [0, 5)
    T8 = np.zeros((128, 8, KBS), np.float32)
    for dd in range(4):
        for eo in range(2):
            for c in range(COUT):
                for q in range(6):
                    jj = 2 * q + eo
                    for di in range(KS):
                        for dj in range(KS):
                            T8[(dd + di) * 16 + jj + dj, dd * 2 + eo, c * 6 + q] = (
                                conv_w[c, di, dj]
                            )
    t8 = np.ascontiguousarray(T8.reshape(128, 8 * KBS)).astype(bf16)

    # fc1 weights to pooled-feature order: block j=(jb*6+g)*2+st, row c*6+q
    # -> original flat feature c*144 + (2g+st)*12 + jb*6 + q
    jv = np.arange(KB)
    jbv, gv, stv = jv // 12, (jv % 12) // 2, jv % 2
    ipv = 2 * gv + stv
    ml = np.arange(KBS)
    cv, qv = ml // 6, ml % 6
    fidx = (
        cv[None, :] * 144 + ipv[:, None] * 12 + jbv[:, None] * 6 + qv[None, :]
    )  # [KB, KBS]
    w1 = fc1_w[:, fidx]                       # [500, KB, KBS]
    w1 = np.ascontiguousarray(w1.transpose(2, 1, 0)).reshape(KBS, KB * FC1_OUT)
    w1 = w1.astype(bf16)

    # conv bias folded into fc1 bias (maxpool commutes with per-channel const)
    cb_vec = np.repeat(conv_b, 144)
    b1p = fc1_b + fc1_w @ cb_vec
    b1 = np.ascontiguousarray(b1p.reshape(MT, MTS).T)

    w2 = np.ascontiguousarray(
        fc2_w.T.reshape(MT, MTS, FC2_OUT).transpose(1, 0, 2)
    ).reshape(MTS, MT * FC2_OUT).astype(bf16)
    b2 = np.ascontiguousarray(fc2_b.reshape(FC2_OUT, 1))
    return t8, w1, b1, w2, b2


# im2col pixel indices: IDX[p, jb, g] = (4g + p//16)*28 + 12jb + (p%16)
_P = np.arange(128)
_IDX = (
    (4 * np.arange(G)[None, None, :] + (_P // 16)[:, None, None]) * W
    + 12 * np.arange(JB)[None, :, None]
    + (_P % 16)[:, None, None]
)  # [128, JB, G]


def _prep_x(x_bf16, nb=256):
    """x_bf16 [npc, 784] bf16 -> xg [128, nbt, NSEG, nb] bf16."""
    npc = x_bf16.shape[0]
    g = x_bf16[:, _IDX.reshape(-1)]           # [npc, 128*JB*G]
    g = g.T.reshape(128, NSEG, npc // nb, nb).transpose(0, 2, 1, 3)
    return np.ascontiguousarray(g)


def _run(inputs, npc=NPC, nb=256, trace=False):
    import ml_dtypes
    from concourse import bass_utils

    key = (npc, nb)
    if key not in _cache:
        _cache[key] = _build(npc, nb)
    nc = _cache[key]

    t8, w1, b1, w2, b2 = _prep_weights(
        inputs["conv_w"], inputs["conv_b"], inputs["fc1_w"],
        inputs["fc1_b"], inputs["fc2_w"], inputs["fc2_b"],
    )
    x = np.asarray(inputs["x"], np.float32).reshape(-1, H * W)
    n_total = x.shape[0]
    assert n_total == NCORES * npc
    xb = x.astype(ml_dtypes.bfloat16)

    in_maps = [
        {
            "xg": _prep_x(xb[i * npc : (i + 1) * npc], nb),
            "t8": t8, "w1": w1, "b1": b1, "w2": w2, "b2": b2,
        }
        for i in range(NCORES)
    ]
    res = bass_utils.run_bass_kernel_spmd(
        nc, in_maps, core_ids=list(range(NCORES)), trace=trace
    )
    out = np.concatenate(
        [np.asarray(res.results[i]["out"], np.float32).T for i in range(NCORES)],
        axis=0,
    )
    return out, res


def kernel(**inputs):
    out, _ = _run(inputs)
    return out


# revision 10
# speedup vs baseline: 1.0895x; 1.0019x over previous
"""Trainium2 Bass kernel for nn_CNNVectorForm (LeNet-style CNN, batch 8192).

Pipeline per core (data-parallel over batch, 1024 images/core):
  conv 5x5 VALID (1->20ch, 28->24) -> 2x2 maxpool -> fc1(2880->500) + relu
  -> fc2(500->10) + softmax

Device formulation (v3 — bf16, coalesced DMA, dense+warm PE):
  * Everything feature-major [features, batch]; batch rides the free dim
    (nb=256 per tile, 4 tiles per core).
  * Inputs, conv/fc weights in bf16 (validated ~4.6e-3 final rel err);
    PSUM accumulation stays fp32.
  * Conv as Toeplitz matmul with K=128 = 8 input rows x 16 cols gathers:
    host im2col expansion is only 1.96x (3.1MB/core in bf16), loaded as
    13 contiguous per-partition DMAs (128 descriptors each) instead of
    96 small gathers - the baseline's DMA-issue serialization starved
    the PE below 1.2GHz.
  * One 8-row gather feeds 4 conv rows = 2 pooled-row "sets"; a set is
    4 matmuls (2 conv rows x 2 col-parities) into 2 one-bank PSUM tiles
    [120, 2, 256] packed so 2x2 maxpool is: scalar copy (PSUM->SBUF),
    DVE max (SBUF,PSUM), DVE bf16 max (halves) -> a1 bf16.  Every 6th
    set swaps work between scalar and DVE to balance both at ~95us.
  * fc1 runs 1:1 interleaved with conv (4+4 matmuls per set, lag 3) so
    the PE never idles; dummy warm-up matmuls during the ~7us framework
    preamble bring the PE HAM clock to 2.4GHz before real work starts.
  * fc1 accumulators: 2 one-bank PSUM tiles [125, 2, 256]; start/stop
    flags are bank-granular (has_written clears whole banks).
  * conv bias folded into the fc1 bias on the host.
  * The relu/fc2/softmax tail of each batch tile is staggered across
    the first sets of the next tile so it hides under conv/fc1; softmax
    row-sum via an all-ones [10,10] matmul, fast-approx reciprocal;
    output [10, npc] is transposed on the host.
  * DMA rings: sync (HWDGE) carries t8 + input; scalar carries the tiny
    biases/fc2; gpsimd (SWDGE) streams the fc1 weights.
"""

import numpy as np

N, H, W = 8192, 28, 28
COUT, KS = 20, 5
NCORES = 8
NPC = N // NCORES        # images per core
G = 6                    # row groups of 8 input rows (stride 4)
JB = 2                   # 16-wide column blocks at offsets 0, 12
NSEG = JB * G            # 12 (jb, g) segments -> 24 pooled-feature blocks
FC1_IN, FC1_OUT, FC2_OUT = 2880, 500, 10
MT, MTS = 4, 125         # fc1 M tiles
KB, KBS = 24, 120        # fc1 K blocks (one per (pooled row, column half))
LAG = 3                  # fc1 trails conv by this many sets
NWARM = 18               # PE warm-up matmuls

_cache = {}


def _build(npc, nb):
    from contextlib import ExitStack

    import concourse.tile as tile
    from concourse import bacc, mybir

    f32 = mybir.dt.float32
    bf16 = mybir.dt.bfloat16
    nbt = npc // nb

    nc = bacc.Bacc(
        "TRN2",
        target_bir_lowering=False,
        debug=False,
        enable_asserts=False,
        num_devices=NCORES,
    )

    # host-im2col'd input: xg[p, jb, g, b] = x[(4g + p//16)*28 + 12jb + p%16, b]
    xg_d = nc.dram_tensor(
        "xg", [128, nbt, NSEG, nb], bf16, kind="ExternalInput"
    ).ap()
    t8_d = nc.dram_tensor("t8", [128, 8 * KBS], bf16, kind="ExternalInput").ap()
    w1_d = nc.dram_tensor("w1", [KBS, KB * FC1_OUT], bf16, kind="ExternalInput").ap()
    b1_d = nc.dram_tensor("b1", [MTS, MT], f32, kind="ExternalInput").ap()
    w2_d = nc.dram_tensor("w2", [MTS, MT * FC2_OUT], bf16, kind="ExternalInput").ap()
    b2_d = nc.dram_tensor("b2", [FC2_OUT, 1], f32, kind="ExternalInput").ap()
    o_d = nc.dram_tensor("out", [FC2_OUT, npc], f32, kind="ExternalOutput").ap()

    with tile.TileContext(nc) as tc, ExitStack() as ctx:
        const = ctx.enter_context(tc.tile_pool(name="const", bufs=1))
        spool = ctx.enter_context(tc.tile_pool(name="spool", bufs=3))
        mpool = ctx.enter_context(tc.tile_pool(name="mpool", bufs=3))
        a1pool = ctx.enter_context(tc.tile_pool(name="a1", bufs=6))
        a2pool = ctx.enter_context(tc.tile_pool(name="a2", bufs=8))
        smpool = ctx.enter_context(tc.tile_pool(name="softmax", bufs=2))
        cpsum = ctx.enter_context(tc.tile_pool(name="cpsum", bufs=4, space="PSUM"))
        fpsum = ctx.enter_context(tc.tile_pool(name="fpsum", bufs=1, space="PSUM"))
        tpsum = ctx.enter_context(tc.tile_pool(name="tpsum", bufs=1, space="PSUM"))

        # --- PE warm-up: HAM releases the 2.4GHz clock only after ~3.4us of
        # sustained matmul activity; burn the framework preamble + first DMA
        # wait on dummies so the real stream starts warm.
        dums = const.tile([128, 384], bf16)
        nc.gpsimd.memset(dums[:], 0.0)
        warmps = tpsum.tile([128, nb], f32, tag="p2f", name="warmps")
        for wi in range(NWARM):
            nc.tensor.matmul(
                warmps[:, :nb], dums[:, 0:128], dums[:, 128 : 128 + nb],
                start=True, stop=True,
            )

        # --- constants / weights -------------------------------------------
        # sync ring: conv stationaries first, then a small first input chunk
        # so conv can start ~10.5us in, then the 12 input segments.
        t8s = const.tile([128, 8 * KBS], bf16)
        nc.sync.dma_start(t8s[:], t8_d[:])
        xfirst = const.tile([128, nb], bf16)
        nc.sync.dma_start(xfirst[:], xg_d[:, 0, 0, :])
        # input: batch-major layout -> one DMA per batch tile with 6KB
        # per-partition contiguous runs (~300GB/s); bt0's whole input is
        # resident by ~11.5us, later tiles stream far ahead of use.
        xbt = []
        for b in range(nbt):
            xt = const.tile([128, NSEG * nb], bf16, name=f"xb{b}")
            if b == 0:
                # bt0 is consumed as it arrives: per-segment DMAs for the
                # first half so the PE never waits on one big transfer
                for k in range(1, 6):
                    nc.sync.dma_start(
                        xt[:, k * nb : (k + 1) * nb], xg_d[:, 0, k, :]
                    )
                nc.sync.dma_start(xt[:, 6 * nb :], xg_d[:, 0, 6:, :])
            else:
                nc.sync.dma_start(xt[:], xg_d[:, b, :, :])
            xbt.append(xt)
        b1t = const.tile([MTS, MT], f32)
        nc.scalar.dma_start(b1t[:], b1_d[:])
        w2t = const.tile([MTS, MT * FC2_OUT], bf16)
        nc.scalar.dma_start(w2t[:], w2_d[:])
        b2t = const.tile([FC2_OUT, 1], f32)
        nc.scalar.dma_start(b2t[:], b2_d[:])
        # gpsimd (SWDGE) ring: the fc1 weight stream; a small first chunk so
        # fc1 j=0 isn't gated on a big transfer.
        w1t = []       # per j-block [KBS, FC1_OUT] views
        w1chunks = [2, 4, 4, 4, 4, 4, 2]
        j0 = 0
        for ci, cn in enumerate(w1chunks):
            wt = const.tile([KBS, cn * FC1_OUT], bf16, name=f"w1c{ci}")
            nc.gpsimd.dma_start(
                wt[:], w1_d[:, j0 * FC1_OUT : (j0 + cn) * FC1_OUT]
            )
            for k in range(cn):
                w1t.append(wt[:, k * FC1_OUT : (k + 1) * FC1_OUT])
            j0 += cn

        ones10 = const.tile([FC2_OUT, FC2_OUT], bf16)
        nc.gpsimd.memset(ones10[:], 1.0)
        # warm the scalar activation table set (exp+relu+copy) while idle
        wout = const.tile([1, 8], f32)
        nc.scalar.activation(wout[:], dums[:1, :8], mybir.ActivationFunctionType.Exp)

        outbuf = const.tile([FC2_OUT, npc], f32)

        def w1_slice(j, mt):
            return w1t[j][:, mt * MTS : (mt + 1) * MTS]

        def fc1_mms(fp, a1, j):
            # fp = (fpA, fpB): two 1-bank tiles, each packing 2 M-tiles.
            # start/stop are bank-wide (has_written clears the whole bank),
            # so only the first/last matmul touching a bank carries the flag.
            for mt in range(MT):
                nc.tensor.matmul(
                    fp[mt // 2][:, mt % 2, :],
                    w1_slice(j, mt),
                    a1[j][:],
                    start=(j == 0 and mt % 2 == 0),
                    stop=(j == KB - 1 and mt % 2 == 1),
                )

        def tail_slots(bt, fp, b0):
            """relu -> fc2 -> softmax for batch tile bt, split into slots
            emitted across the first sets of the next batch tile.  The relu
            slots (0,1) must be emitted before the next tile's fc1 j=0 (the
            fpA/fpB slots are reused, bufs=1)."""
            a2t = [None] * MT
            st8 = {}

            def relu_pair(k):
                def f():
                    for mt in (2 * k, 2 * k + 1):
                        a2 = a2pool.tile(
                            [MTS, nb], bf16, tag="a2", name=f"a2_{bt}_{mt}"
                        )
                        src = fp[mt // 2][:, mt % 2, :]
                        if mt % 2 == 0:
                            nc.scalar.activation(
                                a2[:], src, mybir.ActivationFunctionType.Relu,
                                bias=b1t[:, mt : mt + 1],
                            )
                        else:
                            nc.vector.tensor_scalar(
                                a2[:], src, b1t[:, mt : mt + 1], 0.0,
                                mybir.AluOpType.add, mybir.AluOpType.max,
                            )
                        a2t[mt] = a2
                return f

            def fc2():
                p2f = tpsum.tile([FC2_OUT, nb], f32, tag="p2f", name=f"p2f_{bt}")
                for mt in range(MT):
                    nc.tensor.matmul(
                        p2f[:],
                        w2t[:, mt * FC2_OUT : (mt + 1) * FC2_OUT],
                        a2t[mt][:],
                        start=(mt == 0),
                        stop=(mt == MT - 1),
                    )
                st8["p2f"] = p2f

            def expsum():
                e = smpool.tile([FC2_OUT, nb], bf16, tag="e", name=f"e_{bt}")
                nc.scalar.activation(
                    e[:], st8["p2f"][:], mybir.ActivationFunctionType.Exp,
                    bias=b2t[:, 0:1],
                )
                ssum = tpsum.tile([FC2_OUT, nb], f32, tag="ssum", name=f"ss_{bt}")
                nc.tensor.matmul(ssum[:], ones10[:], e[:], start=True, stop=True)
                st8["e"], st8["ssum"] = e, ssum

            def norm():
                rinv = smpool.tile([FC2_OUT, nb], f32, tag="ri", name=f"ri_{bt}")
                nc.vector.reciprocal_approx_fast(rinv[:], st8["ssum"][:])
                nc.vector.tensor_mul(outbuf[:, b0 : b0 + nb], st8["e"][:], rinv[:])
                nc.sync.dma_start(o_d[:, b0 : b0 + nb], outbuf[:, b0 : b0 + nb])

            return [relu_pair(0), relu_pair(1), fc2, expsum, norm]

        pending = []
        for bt in range(nbt):
            b0 = bt * nb
            a1 = [None] * KB
            fp = (
                fpsum.tile([MTS, 2, nb], f32, tag="fpA", name=f"fpA{bt}"),
                fpsum.tile([MTS, 2, nb], f32, tag="fpB", name=f"fpB{bt}"),
            )
            for idx in range(NSEG):
                for st in range(2):
                    sidx = idx * 2 + st
                    if bt == 0 and idx == 0:
                        rhs = xfirst[:]
                    else:
                        rhs = xbt[bt][:, idx * nb : (idx + 1) * nb]
                    # conv: 4 matmuls -> 2 one-bank PSUM tiles [120, 2, nb]
                    # t0 = conv row 4g+2st (both col parities), t1 = row +1
                    t0 = cpsum.tile([KBS, 2, nb], f32, tag="cps", name=f"t0_{bt}_{sidx}")
                    t1 = cpsum.tile([KBS, 2, nb], f32, tag="cps", name=f"t1_{bt}_{sidx}")
                    for half, tt in ((0, t0), (1, t1)):
                        dd = 2 * st + half
                        for eo in range(2):
                            nc.tensor.matmul(
                                tt[:, eo, :],
                                t8s[:, (dd * 2 + eo) * KBS : (dd * 2 + eo + 1) * KBS],
                                rhs,
                                start=(eo == 0),
                                stop=(eo == 1),
                            )
                    # 2x2 maxpool: max over (t0, t1) x (eo halves).  PSUM
                    # evacuation is split between scalar and DVE; every 6th
                    # set uses 2 scalar copies + a cheap bf16 DVE max instead
                    # of 1 copy + a slow fp32-PSUM DVE max, balancing the
                    # two engines.
                    m = mpool.tile([KBS, 2, nb], bf16, tag="m")
                    if sidx % 6 == 5:
                        s0 = spool.tile([KBS, 2, nb], bf16, tag="s")
                        nc.scalar.copy(s0[:], t0[:])
                        s1 = spool.tile([KBS, 2, nb], bf16, tag="s")
                        nc.scalar.copy(s1[:], t1[:])
                        nc.vector.tensor_max(m[:], s0[:], s1[:])
                    else:
                        s = spool.tile([KBS, 2, nb], f32, tag="sf")
                        nc.scalar.copy(s[:], t0[:])
                        nc.vector.tensor_max(m[:], s[:], t1[:])
                    ab = a1pool.tile([KBS, nb], bf16, tag="a1")
                    nc.vector.tensor_max(ab[:], m[:, 0, :], m[:, 1, :])
                    a1[sidx] = ab
                    # fc1 trails conv so the PE alternates 4 conv + 4 fc1
                    if sidx >= LAG:
                        fc1_mms(fp, a1, sidx - LAG)
                    # previous tile's relu/fc2/softmax, staggered over sets
                    # 1..5 (relu slots land before this tile's fc1 j=0)
                    if pending and 1 <= sidx <= len(pending):
                        pending[sidx - 1]()
                        if sidx == len(pending):
                            pending = []
            for j in range(KB - LAG, KB):
                fc1_mms(fp, a1, j)
            pending = tail_slots(bt, fp, b0)
        for f in pending:
            f()

    nc.compile()
    return nc


def _prep_weights(conv_w, conv_b, fc1_w, fc1_b, fc2_w, fc2_b):
    import ml_dtypes

    bf16 = ml_dtypes.bfloat16
    conv_w = np.asarray(conv_w, np.float32).reshape(COUT, KS, KS)
    conv_b = np.asarray(conv_b, np.float32)
    fc1_w = np.asarray(fc1_w, np.float32)
    fc1_b = np.asarray(fc1_b, np.float32)
    fc2_w = np.asarray(fc2_w, np.float32)
    fc2_b = np.asarray(fc2_b, np.float32)

    # conv stationaries: T8[p = di8*16+jjp, k = dd*2+eo, m = c*6+q]
    #  = conv_w[c, di8-dd, jjp-(2q+eo)] where both offsets are in [0, 5)
    T8 = np.zeros((128, 8, KBS), np.float32)
    for dd in range(4):
        for eo in range(2):
            for c in range(COUT):
                for q in range(6):
                    jj = 2 * q + eo
                    for di in range(KS):
                        for dj in range(KS):
                            T8[(dd + di) * 16 + jj + dj, dd * 2 + eo, c * 6 + q] = (
                                conv_w[c, di, dj]
                            )
    t8 = np.ascontiguousarray(T8.reshape(128, 8 * KBS)).astype(bf16)

    # fc1 weights to pooled-feature order: block j=(jb*6+g)*2+st, row c*6+q
    # -> original flat feature c*144 + (2g+st)*12 + jb*6 + q
    jv = np.arange(KB)
    jbv, gv, stv = jv // 12, (jv % 12) // 2, jv % 2
    ipv = 2 * gv + stv
    ml = np.arange(KBS)
    cv, qv = ml // 6, ml % 6
    fidx = (
        cv[None, :] * 144 + ipv[:, None] * 12 + jbv[:, None] * 6 + qv[None, :]
    )  # [KB, KBS]
    w1 = fc1_w[:, fidx]                       # [500, KB, KBS]
    w1 = np.ascontiguousarray(w1.transpose(2, 1, 0)).reshape(KBS, KB * FC1_OUT)
    w1 = w1.astype(bf16)

    # conv bias folded into fc1 bias (maxpool commutes with per-channel const)
    cb_vec = np.repeat(conv_b, 144)
    b1p = fc1_b + fc1_w @ cb_vec
    b1 = np.ascontiguousarray(b1p.reshape(MT, MTS).T)

    w2 = np.ascontiguousarray(
        fc2_w.T.reshape(MT, MTS, FC2_OUT).transpose(1, 0, 2)
    ).reshape(MTS, MT * FC2_OUT).astype(bf16)
    b2 = np.ascontiguousarray(fc2_b.reshape(FC2_OUT, 1))
    return t8, w1, b1, w2, b2


# im2col pixel indices: IDX[p, jb, g] = (4g + p//16)*28 + 12jb + (p%16)
_P = np.arange(128)
_IDX = (
    (4 * np.arange(G)[None, None, :] + (_P // 16)[:, None, None]) * W
    + 12 * np.arange(JB)[None, :, None]
    + (_P % 16)[:, None, None]
)  # [128, JB, G]


def _prep_x(x_bf16, nb=256):
    """x_bf16 [npc, 784] bf16 -> xg [128, nbt, NSEG, nb] bf16."""
    npc = x_bf16.shape[0]
    g = x_bf16[:, _IDX.reshape(-1)]           # [npc, 128*JB*G]
    g = g.T.reshape(128, NSEG, npc // nb, nb).transpose(0, 2, 1, 3)
    return np.ascontiguousarray(g)


def _run(inputs, npc=NPC, nb=256, trace=False):
    import ml_dtypes
    from concourse import bass_utils

    key = (npc, nb)
    if key not in _cache:
        _cache[key] = _build(npc, nb)
    nc = _cache[key]

    t8, w1, b1, w2, b2 = _prep_weights(
        inputs["conv_w"], inputs["conv_b"], inputs["fc1_w"],
        inputs["fc1_b"], inputs["fc2_w"], inputs["fc2_b"],
    )
    x = np.asarray(inputs["x"], np.float32).reshape(-1, H * W)
    n_total = x.shape[0]
    assert n_total == NCORES * npc
    xb = x.astype(ml_dtypes.bfloat16)

    in_maps = [
        {
            "xg": _prep_x(xb[i * npc : (i + 1) * npc], nb),
            "t8": t8, "w1": w1, "b1": b1, "w2": w2, "b2": b2,
        }
        for i in range(NCORES)
    ]
    res = bass_utils.run_bass_kernel_spmd(
        nc, in_maps, core_ids=list(range(NCORES)), trace=trace
    )
    out = np.concatenate(
        [np.asarray(res.results[i]["out"], np.float32).T for i in range(NCORES)],
        axis=0,
    )
    return out, res


def kernel(**inputs):
    out, _ = _run(inputs)
    return out


# revision 11
# speedup vs baseline: 1.1116x; 1.0202x over previous
"""Trainium2 Bass kernel for nn_CNNVectorForm (LeNet-style CNN, batch 8192).

Pipeline per core (data-parallel over batch, 1024 images/core):
  conv 5x5 VALID (1->20ch, 28->24) -> 2x2 maxpool -> fc1(2880->500) + relu
  -> fc2(500->10) + softmax

Device formulation (v3 — bf16, coalesced DMA, dense+warm PE):
  * Everything feature-major [features, batch]; batch rides the free dim
    (nb=256 per tile, 4 tiles per core).
  * Inputs, conv/fc weights in bf16 (validated ~4.6e-3 final rel err);
    PSUM accumulation stays fp32.
  * Conv as Toeplitz matmul with K=128 = 8 input rows x 16 cols gathers:
    host im2col expansion is only 1.96x (3.1MB/core in bf16), loaded as
    13 contiguous per-partition DMAs (128 descriptors each) instead of
    96 small gathers - the baseline's DMA-issue serialization starved
    the PE below 1.2GHz.
  * One 8-row gather feeds 4 conv rows = 2 pooled-row "sets"; a set is
    4 matmuls (2 conv rows x 2 col-parities) into 2 one-bank PSUM tiles
    [120, 2, 256] packed so 2x2 maxpool is: scalar copy (PSUM->SBUF),
    DVE max (SBUF,PSUM), DVE bf16 max (halves) -> a1 bf16.  Every 6th
    set swaps work between scalar and DVE to balance both at ~95us.
  * fc1 runs 1:1 interleaved with conv (4+4 matmuls per set, lag 3) so
    the PE never idles; dummy warm-up matmuls during the ~7us framework
    preamble bring the PE HAM clock to 2.4GHz before real work starts.
  * fc1 accumulators: 2 one-bank PSUM tiles [125, 2, 256]; start/stop
    flags are bank-granular (has_written clears whole banks).
  * conv bias folded into the fc1 bias on the host.
  * The relu/fc2/softmax tail of each batch tile is staggered across
    the first sets of the next tile so it hides under conv/fc1; softmax
    row-sum via an all-ones [10,10] matmul, fast-approx reciprocal;
    output [10, npc] is transposed on the host.
  * DMA rings: sync (HWDGE) carries t8 + input; scalar carries the tiny
    biases/fc2; gpsimd (SWDGE) streams the fc1 weights.
"""

import numpy as np

N, H, W = 8192, 28, 28
COUT, KS = 20, 5
NCORES = 8
NPC = N // NCORES        # images per core
G = 6                    # row groups of 8 input rows (stride 4)
JB = 2                   # 16-wide column blocks at offsets 0, 12
NSEG = JB * G            # 12 (jb, g) segments -> 24 pooled-feature blocks
FC1_IN, FC1_OUT, FC2_OUT = 2880, 500, 10
MT, MTS = 4, 125         # fc1 M tiles
KB, KBS = 24, 120        # fc1 K blocks (one per (pooled row, column half))
LAG = 3                  # fc1 trails conv by this many sets
NWARM = 18               # PE warm-up matmuls

_cache = {}


def _build(npc, nb):
    from contextlib import ExitStack

    import concourse.tile as tile
    from concourse import bacc, mybir

    f32 = mybir.dt.float32
    bf16 = mybir.dt.bfloat16
    nbt = npc // nb

    nc = bacc.Bacc(
        "TRN2",
        target_bir_lowering=False,
        debug=False,
        enable_asserts=False,
        num_devices=NCORES,
    )

    # host-im2col'd input: xg[p, jb, g, b] = x[(4g + p//16)*28 + 12jb + p%16, b]
    xg_d = nc.dram_tensor(
        "xg", [128, nbt, NSEG, nb], bf16, kind="ExternalInput"
    ).ap()
    t8_d = nc.dram_tensor("t8", [128, 8 * KBS], bf16, kind="ExternalInput").ap()
    w1_d = nc.dram_tensor("w1", [KBS, KB * FC1_OUT], bf16, kind="ExternalInput").ap()
    b1_d = nc.dram_tensor("b1", [MTS, MT], f32, kind="ExternalInput").ap()
    w2_d = nc.dram_tensor("w2", [MTS, MT * FC2_OUT], bf16, kind="ExternalInput").ap()
    b2_d = nc.dram_tensor("b2", [FC2_OUT, 1], f32, kind="ExternalInput").ap()
    o_d = nc.dram_tensor("out", [FC2_OUT, npc], f32, kind="ExternalOutput").ap()

    with tile.TileContext(nc) as tc, ExitStack() as ctx:
        const = ctx.enter_context(tc.tile_pool(name="const", bufs=1))
        spool = ctx.enter_context(tc.tile_pool(name="spool", bufs=3))
        mpool = ctx.enter_context(tc.tile_pool(name="mpool", bufs=3))
        a1pool = ctx.enter_context(tc.tile_pool(name="a1", bufs=6))
        a2pool = ctx.enter_context(tc.tile_pool(name="a2", bufs=8))
        smpool = ctx.enter_context(tc.tile_pool(name="softmax", bufs=2))
        cpsum = ctx.enter_context(tc.tile_pool(name="cpsum", bufs=4, space="PSUM"))
        fpsum = ctx.enter_context(tc.tile_pool(name="fpsum", bufs=1, space="PSUM"))
        tpsum = ctx.enter_context(tc.tile_pool(name="tpsum", bufs=1, space="PSUM"))

        # --- PE warm-up: HAM releases the 2.4GHz clock only after ~3.4us of
        # sustained matmul activity; burn the framework preamble + first DMA
        # wait on dummies so the real stream starts warm.
        dums = const.tile([128, 384], bf16)
        nc.gpsimd.memset(dums[:], 0.0)
        warmps = tpsum.tile([128, nb], f32, tag="p2f", name="warmps")
        for wi in range(NWARM):
            nc.tensor.matmul(
                warmps[:, :nb], dums[:, 0:128], dums[:, 128 : 128 + nb],
                start=True, stop=True,
            )

        # --- constants / weights -------------------------------------------
        # sync ring: conv stationaries first, then a small first input chunk
        # so conv can start ~10.5us in, then the 12 input segments.
        t8s = const.tile([128, 8 * KBS], bf16)
        nc.sync.dma_start(t8s[:], t8_d[:])
        xfirst = const.tile([128, nb], bf16)
        nc.sync.dma_start(xfirst[:], xg_d[:, 0, 0, :])
        # input: batch-major layout -> one DMA per batch tile with 6KB
        # per-partition contiguous runs (~300GB/s); bt0's whole input is
        # resident by ~11.5us, later tiles stream far ahead of use.
        xbt = []
        for b in range(nbt):
            xt = const.tile([128, NSEG * nb], bf16, name=f"xb{b}")
            if b == 0:
                # bt0 is consumed as it arrives: per-segment DMAs for the
                # first half so the PE never waits on one big transfer
                for k in range(1, 7):
                    nc.sync.dma_start(
                        xt[:, k * nb : (k + 1) * nb], xg_d[:, 0, k, :]
                    )
                nc.sync.dma_start(xt[:, 7 * nb :], xg_d[:, 0, 7:, :])
            else:
                nc.sync.dma_start(xt[:], xg_d[:, b, :, :])
            xbt.append(xt)
        b1t = const.tile([MTS, MT], f32)
        nc.scalar.dma_start(b1t[:], b1_d[:])
        w2t = const.tile([MTS, MT * FC2_OUT], bf16)
        nc.scalar.dma_start(w2t[:], w2_d[:])
        b2t = const.tile([FC2_OUT, 1], f32)
        nc.scalar.dma_start(b2t[:], b2_d[:])
        # gpsimd (SWDGE) ring: the fc1 weight stream; a small first chunk so
        # fc1 j=0 isn't gated on a big transfer.
        w1t = []       # per j-block [KBS, FC1_OUT] views
        w1chunks = [2, 2, 2, 3, 3, 4, 4, 4]
        j0 = 0
        for ci, cn in enumerate(w1chunks):
            wt = const.tile([KBS, cn * FC1_OUT], bf16, name=f"w1c{ci}")
            nc.gpsimd.dma_start(
                wt[:], w1_d[:, j0 * FC1_OUT : (j0 + cn) * FC1_OUT]
            )
            for k in range(cn):
                w1t.append(wt[:, k * FC1_OUT : (k + 1) * FC1_OUT])
            j0 += cn

        ones10 = const.tile([FC2_OUT, FC2_OUT], bf16)
        nc.gpsimd.memset(ones10[:], 1.0)
        # warm the scalar activation table set (exp+relu+copy) while idle
        wout = const.tile([1, 8], f32)
        nc.scalar.activation(wout[:], dums[:1, :8], mybir.ActivationFunctionType.Exp)

        outbuf = const.tile([FC2_OUT, npc], f32)

        def w1_slice(j, mt):
            return w1t[j][:, mt * MTS : (mt + 1) * MTS]

        def fc1_mms(fp, a1, j):
            # fp = (fpA, fpB): two 1-bank tiles, each packing 2 M-tiles.
            # start/stop are bank-wide (has_written clears the whole bank),
            # so only the first/last matmul touching a bank carries the flag.
            for mt in range(MT):
                nc.tensor.matmul(
                    fp[mt // 2][:, mt % 2, :],
                    w1_slice(j, mt),
                    a1[j][:],
                    start=(j == 0 and mt % 2 == 0),
                    stop=(j == KB - 1 and mt % 2 == 1),
                )

        def tail_slots(bt, fp, b0):
            """relu -> fc2 -> softmax for batch tile bt, split into slots
            emitted across the first sets of the next batch tile.  The relu
            slots (0,1) must be emitted before the next tile's fc1 j=0 (the
            fpA/fpB slots are reused, bufs=1)."""
            a2t = [None] * MT
            st8 = {}

            def relu_pair(k):
                def f():
                    for mt in (2 * k, 2 * k + 1):
                        a2 = a2pool.tile(
                            [MTS, nb], bf16, tag="a2", name=f"a2_{bt}_{mt}"
                        )
                        src = fp[mt // 2][:, mt % 2, :]
                        if mt % 2 == 0:
                            nc.scalar.activation(
                                a2[:], src, mybir.ActivationFunctionType.Relu,
                                bias=b1t[:, mt : mt + 1],
                            )
                        else:
                            nc.vector.tensor_scalar(
                                a2[:], src, b1t[:, mt : mt + 1], 0.0,
                                mybir.AluOpType.add, mybir.AluOpType.max,
                            )
                        a2t[mt] = a2
                return f

            def fc2():
                p2f = tpsum.tile([FC2_OUT, nb], f32, tag="p2f", name=f"p2f_{bt}")
                for mt in range(MT):
                    nc.tensor.matmul(
                        p2f[:],
                        w2t[:, mt * FC2_OUT : (mt + 1) * FC2_OUT],
                        a2t[mt][:],
                        start=(mt == 0),
                        stop=(mt == MT - 1),
                    )
                st8["p2f"] = p2f

            def expsum():
                e = smpool.tile([FC2_OUT, nb], bf16, tag="e", name=f"e_{bt}")
                nc.scalar.activation(
                    e[:], st8["p2f"][:], mybir.ActivationFunctionType.Exp,
                    bias=b2t[:, 0:1],
                )
                ssum = tpsum.tile([FC2_OUT, nb], f32, tag="ssum", name=f"ss_{bt}")
                nc.tensor.matmul(ssum[:], ones10[:], e[:], start=True, stop=True)
                st8["e"], st8["ssum"] = e, ssum

            def norm():
                rinv = smpool.tile([FC2_OUT, nb], f32, tag="ri", name=f"ri_{bt}")
                nc.vector.reciprocal_approx_fast(rinv[:], st8["ssum"][:])
                nc.vector.tensor_mul(outbuf[:, b0 : b0 + nb], st8["e"][:], rinv[:])
                nc.sync.dma_start(o_d[:, b0 : b0 + nb], outbuf[:, b0 : b0 + nb])

            return [relu_pair(0), relu_pair(1), fc2, expsum, norm]

        def tail_final(bt, fp, b0):
            # exposed tail of the last batch tile: pipeline two half-batches
            # so the relu/fc2/exp/sum/recip chains overlap across engines
            hw2 = nb // 2
            a2t = {}
            for h in range(2):
                hs = slice(h * hw2, (h + 1) * hw2)
                for mt in range(MT):
                    a2 = a2pool.tile([MTS, hw2], bf16, tag="a2f",
                                     name=f"a2f_{h}_{mt}")
                    src_ = fp[mt // 2][:, mt % 2, hs]
                    if mt % 2 == 0:
                        nc.scalar.activation(
                            a2[:], src_, mybir.ActivationFunctionType.Relu,
                            bias=b1t[:, mt : mt + 1],
                        )
                    else:
                        nc.vector.tensor_scalar(
                            a2[:], src_, b1t[:, mt : mt + 1], 0.0,
                            mybir.AluOpType.add, mybir.AluOpType.max,
                        )
                    a2t[(h, mt)] = a2
                p2f = tpsum.tile([FC2_OUT, hw2], f32, tag="p2f",
                                 name=f"p2ff_{h}")
                for mt in range(MT):
                    nc.tensor.matmul(
                        p2f[:],
                        w2t[:, mt * FC2_OUT : (mt + 1) * FC2_OUT],
                        a2t[(h, mt)][:],
                        start=(mt == 0),
                        stop=(mt == MT - 1),
                    )
                e = smpool.tile([FC2_OUT, hw2], bf16, tag="e", name=f"ef_{h}")
                nc.scalar.activation(
                    e[:], p2f[:], mybir.ActivationFunctionType.Exp,
                    bias=b2t[:, 0:1],
                )
                ssum = tpsum.tile([FC2_OUT, hw2], f32, tag="ssum",
                                  name=f"ssf_{h}")
                nc.tensor.matmul(ssum[:], ones10[:], e[:], start=True, stop=True)
                rinv = smpool.tile([FC2_OUT, hw2], f32, tag="ri", name=f"rif_{h}")
                nc.vector.reciprocal_approx_fast(rinv[:], ssum[:])
                c0 = b0 + h * hw2
                nc.vector.tensor_mul(outbuf[:, c0 : c0 + hw2], e[:], rinv[:])
                nc.sync.dma_start(
                    o_d[:, c0 : c0 + hw2], outbuf[:, c0 : c0 + hw2]
                )

        pending = []
        for bt in range(nbt):
            b0 = bt * nb
            a1 = [None] * KB
            fp = (
                fpsum.tile([MTS, 2, nb], f32, tag="fpA", name=f"fpA{bt}"),
                fpsum.tile([MTS, 2, nb], f32, tag="fpB", name=f"fpB{bt}"),
            )
            for idx in range(NSEG):
                for st in range(2):
                    sidx = idx * 2 + st
                    if bt == 0 and idx == 0:
                        rhs = xfirst[:]
                    else:
                        rhs = xbt[bt][:, idx * nb : (idx + 1) * nb]
                    # conv: 4 matmuls -> 2 one-bank PSUM tiles [120, 2, nb]
                    # t0 = conv row 4g+2st (both col parities), t1 = row +1
                    t0 = cpsum.tile([KBS, 2, nb], f32, tag="cps", name=f"t0_{bt}_{sidx}")
                    t1 = cpsum.tile([KBS, 2, nb], f32, tag="cps", name=f"t1_{bt}_{sidx}")
                    for half, tt in ((0, t0), (1, t1)):
                        dd = 2 * st + half
                        for eo in range(2):
                            nc.tensor.matmul(
                                tt[:, eo, :],
                                t8s[:, (dd * 2 + eo) * KBS : (dd * 2 + eo + 1) * KBS],
                                rhs,
                                start=(eo == 0),
                                stop=(eo == 1),
                            )
                    # 2x2 maxpool: max over (t0, t1) x (eo halves).  PSUM
                    # evacuation is split between scalar and DVE; every 6th
                    # set uses 2 scalar copies + a cheap bf16 DVE max instead
                    # of 1 copy + a slow fp32-PSUM DVE max, balancing the
                    # two engines.
                    m = mpool.tile([KBS, 2, nb], bf16, tag="m")
                    if sidx % 6 == 5:
                        s0 = spool.tile([KBS, 2, nb], bf16, tag="s")
                        nc.scalar.copy(s0[:], t0[:])
                        s1 = spool.tile([KBS, 2, nb], bf16, tag="s")
                        nc.scalar.copy(s1[:], t1[:])
                        nc.vector.tensor_max(m[:], s0[:], s1[:])
                    else:
                        s = spool.tile([KBS, 2, nb], f32, tag="sf")
                        nc.scalar.copy(s[:], t0[:])
                        nc.vector.tensor_max(m[:], s[:], t1[:])
                    ab = a1pool.tile([KBS, nb], bf16, tag="a1")
                    nc.vector.tensor_max(ab[:], m[:, 0, :], m[:, 1, :])
                    a1[sidx] = ab
                    # fc1 trails conv so the PE alternates 4 conv + 4 fc1
                    if sidx >= LAG:
                        fc1_mms(fp, a1, sidx - LAG)
                    # previous tile's relu/fc2/softmax, staggered over sets
                    # 1..5 (relu slots land before this tile's fc1 j=0)
                    if pending and 1 <= sidx <= len(pending):
                        pending[sidx - 1]()
                        if sidx == len(pending):
                            pending = []
            for j in range(KB - LAG, KB):
                fc1_mms(fp, a1, j)
            if bt < nbt - 1:
                pending = tail_slots(bt, fp, b0)
            else:
                tail_final(bt, fp, b0)

    nc.compile()
    return nc


def _prep_weights(conv_w, conv_b, fc1_w, fc1_b, fc2_w, fc2_b):
    import ml_dtypes

    bf16 = ml_dtypes.bfloat16
    conv_w = np.asarray(conv_w, np.float32).reshape(COUT, KS, KS)
    conv_b = np.asarray(conv_b, np.float32)
    fc1_w = np.asarray(fc1_w, np.float32)
    fc1_b = np.asarray(fc1_b, np.float32)
    fc2_w = np.asarray(fc2_w, np.float32)
    fc2_b = np.asarray(fc2_b, np.float32)

    # conv stationaries: T8[p = di8*16+jjp, k = dd*2+eo, m = c*6+q]
    #  = conv_w[c, di8-dd, jjp-(2q+eo)] where both offsets are in [0, 5)
    T8 = np.zeros((128, 8, KBS), np.float32)
    for dd in range(4):
        for eo in range(2):
            for c in range(COUT):
                for q in range(6):
                    jj = 2 * q + eo
                    for di in range(KS):
                        for dj in range(KS):
                            T8[(dd + di) * 16 + jj + dj, dd * 2 + eo, c * 6 + q] = (
                                conv_w[c, di, dj]
                            )
    t8 = np.ascontiguousarray(T8.reshape(128, 8 * KBS)).astype(bf16)

    # fc1 weights to pooled-feature order: block j=(jb*6+g)*2+st, row c*6+q
    # -> original flat feature c*144 + (2g+st)*12 + jb*6 + q
    jv = np.arange(KB)
    jbv, gv, stv = jv // 12, (jv % 12) // 2, jv % 2
    ipv = 2 * gv + stv
    ml = np.arange(KBS)
    cv, qv = ml // 6, ml % 6
    fidx = (
        cv[None, :] * 144 + ipv[:, None] * 12 + jbv[:, None] * 6 + qv[None, :]
    )  # [KB, KBS]
    w1 = fc1_w[:, fidx]                       # [500, KB, KBS]
    w1 = np.ascontiguousarray(w1.transpose(2, 1, 0)).reshape(KBS, KB * FC1_OUT)
    w1 = w1.astype(bf16)

    # conv bias folded into fc1 bias (maxpool commutes with per-channel const)
    cb_vec = np.repeat(conv_b, 144)
    b1p = fc1_b + fc1_w @ cb_vec
    b1 = np.ascontiguousarray(b1p.reshape(MT, MTS).T)

    w2 = np.ascontiguousarray(
        fc2_w.T.reshape(MT, MTS, FC2_OUT).transpose(1, 0, 2)
    ).reshape(MTS, MT * FC2_OUT).astype(bf16)
    b2 = np.ascontiguousarray(fc2_b.reshape(FC2_OUT, 1))
    return t8, w1, b1, w2, b2


# im2col pixel indices: IDX[p, jb, g] = (4g + p//16)*28 + 12jb + (p%16)
_P = np.arange(128)
_IDX = (
    (4 * np.arange(G)[None, None, :] + (_P // 16)[:, None, None]) * W
    + 12 * np.arange(JB)[None, :, None]
    + (_P % 16)[:, None, None]
)  # [128, JB, G]


def _prep_x(x_bf16, nb=256):
    """x_bf16 [npc, 784] bf16 -> xg [128, nbt, NSEG, nb] bf16."""
    npc = x_bf16.shape[0]
    g = x_bf16[:, _IDX.reshape(-1)]           # [npc, 128*JB*G]
    g = g.T.reshape(128, NSEG, npc // nb, nb).transpose(0, 2, 1, 3)
    return np.ascontiguousarray(g)


def _run(inputs, npc=NPC, nb=256, trace=False):
    import ml_dtypes
    from concourse import bass_utils

    key = (npc, nb)
    if key not in _cache:
        _cache[key] = _build(npc, nb)
    nc = _cache[key]

    t8, w1, b1, w2, b2 = _prep_weights(
        inputs["conv_w"], inputs["conv_b"], inputs["fc1_w"],
        inputs["fc1_b"], inputs["fc2_w"], inputs["fc2_b"],
    )
    x = np.asarray(inputs["x"], np.float32).reshape(-1, H * W)
    n_total = x.shape[0]
    assert n_total == NCORES * npc
    xb = x.astype(ml_dtypes.bfloat16)

    in_maps = [
        {
            "xg": _prep_x(xb[i * npc : (i + 1) * npc], nb),
            "t8": t8, "w1": w1, "b1": b1, "w2": w2, "b2": b2,
        }
        for i in range(NCORES)
    ]
    res = bass_utils.run_bass_kernel_spmd(
        nc, in_maps, core_ids=list(range(NCORES)), trace=trace
    )
    out = np.concatenate(
        [np.asarray(res.results[i]["out"], np.float32).T for i in range(NCORES)],
        axis=0,
    )
    return out, res


def kernel(**inputs):
    out, _ = _run(inputs)
    return out
